# revision 88
# baseline (speedup 1.0000x reference)
"""Trainium2 Bass kernel: multi-head attention (B=2, S=2048, D=1024, H=16,
d_k=64) with RoPE and masked softmax, sharded over 8 NeuronCores as
(batch x head-group): core = b*4 + g handles batch b, heads [4g, 4g+4).

Per-core device program, engine-budget driven (sim: PE ~132us busy, ACT ~95,
DVE ~75, Pool ~50; total ~161us). The softmax exp can only run on ACT and is
the attention-phase ceiling, so the schedule is built to start the exp
stream early and keep it dense:

  1. Inputs: x and QKV weights ship bf16 (same 1 cycle/row PE rate as
     fp32r, half the DMA bytes); scores/PV/Wo stay fp32r; partial outputs
     ship bf16 and the host sums the 4 group partials in f32. v ships
     host-pre-swizzled (xvS) so each s-tile column slab is one
     contiguous-descriptor DMA.
  2. Q-t0/K-t0 (heads 0,1) use a DUAL projection: W x and (P W) x with the
     rotate-half permutation P folded into a second host-side weight copy
     (sign in the sin table), so RoPE is just main*cos + rot*sin read
     straight from PSUM — no rotate DMAs, whose tile-WAW chains cost
     ~3.2us per link. Q-t1/K-t1 project classically; their rotate chains
     and DVE muls run under early attention, finishing right as heads 2,3
     start (head order: t0 pair of both q-chunks first).
  3. The V projection is fused into the attention stream: head 0 of each
     q-chunk emits one V s-tile (SP-issued slab load, 8 matmuls into a
     PSUM tile borrowed from the ctx pool's idle slot, ACT evac) two
     k-tiles ahead of the scores that consume it, so V never sits on the
     pre-attention critical path. A ones column per 65-wide V head block
     makes the PV matmul emit softmax denominators for free (M=65).
  4. scores^T = K_h(dk x S) x Q_h -> [k, q] PSUM tiles, causal sub-ranges
     chosen so the moving dim stays >= 256 (fp32r full-rate threshold);
     exp on ACT over [ax:] only; DVE multiplies the diagonal 128-block
     (plus the <=128-col gap kept for matmul width) of the exp OUTPUT by a
     host 0/1 triangle in SBUF — no -1e9 memsets or PSUM bias adds. ctx^T
     accumulates over k-tiles at sub-bank ranges with causal skipping; the
     kt loop is software-pipelined (PV lags 3) so the in-order PE stream
     never parks behind exp.
  5. 1/sumexp: DVE reciprocal, SP-issued 0-stride-source SBUF->SBUF DMA
     broadcasts it across 64 partitions, DVE applies it while evacuating
     ctx. W_o runs after both attention chunks (scheduler fills exp-bound
     gaps); evacs alternate DVE/ACT, Pool-issued bf16 stores.

Engine roles: SP = input loads + rb broadcast; Pool = const loads, t1
rotate DMAs, output stores (nothing attention-critical); ACT = exp, Q/K
evacs (pre-attention), V evacs, half the Wo evacs; DVE = everything else.
Wait counts >1 are hoisted onto single-wait no-ops after scheduling
(walrus codegen limitation)."""
import sys

sys.path.insert(0, "/opt/trn_rl_repo")

from contextlib import ExitStack

import numpy as np

import concourse.bass as bass
import concourse.mybir as mybir
import concourse.tile as tile
FP = mybir.dt.float32
FPR = mybir.dt.float32r
BF = mybir.dt.bfloat16
EXP = mybir.ActivationFunctionType.Exp

D = 1024        # d_model
S = 2048        # sequence length
NB = 2          # batches
HPG = 4         # heads per group (= per core)
DK = 64         # head dim
F = HPG * DK    # 256 = group feature width
KT = D // 128   # 8 contraction tiles for projections
ST = S // 128   # 16 seq tiles
QCW = 1024      # q-chunk width (= 2 PSUM banks)
NQC = S // QCW  # 2
NEG = -1e9

_nc_cache = {}

# rotate-half column permutation for the dual K projection: within each
# head's 64-wide dk block, swap the 32-halves (sign is folded into sin)
_ROT_PERM = np.array([64 * (c // 64) + ((c % 64) ^ 32) for c in range(F)])


def _mm(nc, out, lhsT, rhs, **kw):
    nc.tensor.matmul(out, lhsT, rhs, **kw)


def _hoist_waits(nc):
    """Several walrus codegen structs (fused-LDW fp32/fp32r matmul, pseudo
    direct2d DMA, ...) only have room for a single sync wait. Hoist every
    limited instruction's waits (when >1) onto same-engine no-ops inserted
    just before it."""
    f = nc.m.functions[0]

    def engine_builder(eng):
        return {
            mybir.EngineType.PE: nc.tensor,
            mybir.EngineType.DVE: nc.vector,
            mybir.EngineType.Activation: nc.scalar,
            mybir.EngineType.Pool: nc.gpsimd,
            mybir.EngineType.SP: nc.sync,
        }[eng]

    def fresh_nop(eng):
        inst = engine_builder(eng).nop().ins
        for b in f.blocks:
            for i, x in enumerate(b.instructions):
                if x is inst:
                    del b.instructions[i]
                    return inst
        raise RuntimeError("created nop not found in any block")

    total = 0
    for blk in f.blocks:
        out = []
        for inst in blk.instructions:
            si = inst.sync_info
            if si is not None and len(si.on_wait) > 1:
                for w in si.on_wait[:-1]:
                    nop = fresh_nop(inst.engine)
                    nop.sync_info = mybir.SyncInfo(on_wait=[w], on_update=[])
                    out.append(nop)
                    total += 1
                inst.sync_info = mybir.SyncInfo(on_wait=[si.on_wait[-1]],
                                                on_update=list(si.on_update))
            out.append(inst)
        blk.instructions[:] = out
    return total


def _causal_ranges(qc, kt):
    """Per (q-chunk, k-tile) causal sub-ranges.

    Returns (j0, a0, a1) where j0 is the first unmasked q column, and
    [a0:512] / [a1:1024] are the bank-0/1 compute ranges (a >= 512 or
    >= 1024 means the bank is skipped). a is pulled below j0 only to keep
    the matmul moving dim >= 256 (fp32r full-rate threshold); the [a:j0)
    gap is zeroed in e_t by Pool."""
    j0 = max(0, kt * 128 - qc * QCW)
    if j0 < 512:
        a0 = j0 if 512 - j0 >= 256 else 256
    else:
        a0 = 512  # bank 0 fully masked
    j1 = max(j0, 512)
    if j1 < 1024:
        a1 = j1 if 1024 - j1 >= 256 else 768
    else:
        a1 = 1024
    return j0, a0, a1


def build_nc(mask_mode):
    """mask_mode: 'causal' | 'full' | 'general'."""
    assert mask_mode in ("causal", "full", "general")
    nc = bass.Bass("TRN2", target_bir_lowering=False, debug=False, num_devices=8)

    # x and the QKV projection weights ship as bf16: the projection matmuls
    # run at the same PE rate as fp32r but input DMA bytes halve (phase 1 is
    # DMA-bandwidth-bound at fp32); everything downstream stays fp32.
    xqT = nc.dram_tensor("xqT", [D, S], BF, kind="ExternalInput").ap()
    xkT = nc.dram_tensor("xkT", [D, S], BF, kind="ExternalInput").ap()
    # v input pre-swizzled host-side so each s-tile's column slab
    # [128, KT*128] is one contiguous DMA (2KB/partition descriptors)
    xvS = nc.dram_tensor("xvS", [ST * 128, KT * 128], BF, kind="ExternalInput").ap()
    wqT = nc.dram_tensor("wqT", [D, F], BF, kind="ExternalInput").ap()
    wkT = nc.dram_tensor("wkT", [D, F], BF, kind="ExternalInput").ap()
    # rotate-half-permuted Q/K weights (sign lives in the sin table)
    wqrT = nc.dram_tensor("wqrT", [D, F], BF, kind="ExternalInput").ap()
    wkrT = nc.dram_tensor("wkrT", [D, F], BF, kind="ExternalInput").ap()
    wvT = nc.dram_tensor("wvT", [D, F], BF, kind="ExternalInput").ap()
    woT = nc.dram_tensor("woT", [F, D], FPR, kind="ExternalInput").ap()
    cosd = nc.dram_tensor("cosS", [128, S], FP, kind="ExternalInput").ap()
    sind = nc.dram_tensor("sinS", [128, S], FP, kind="ExternalInput").ap()
    if mask_mode == "general":
        biasT = nc.dram_tensor("biasT", [S, S], FP, kind="ExternalInput").ap()
    if mask_mode == "causal":
        # [128, 256] 0/1 mask: zero left half, lower-triangle right half —
        # one Pool multiply masks both the diagonal block and the <=128-col
        # gap kept only for matmul width
        triD = nc.dram_tensor("triD", [128, 256], FP, kind="ExternalInput").ap()
    # partial output ships bf16 (host sums the 4 group partials in f32)
    outp = nc.dram_tensor("outp", [S, D], BF, kind="ExternalOutput").ap()

    with tile.TileContext(nc) as tc, ExitStack() as ctx:
        const = ctx.enter_context(tc.tile_pool(name="const", bufs=1))
        qk = ctx.enter_context(tc.tile_pool(name="qk", bufs=1))

        wq_sb = const.tile([128, KT * F], BF)
        wk_sb = const.tile([128, KT * F], BF)
        wqr_sb = const.tile([128, KT * F], BF)
        wkr_sb = const.tile([128, KT * F], BF)
        wv_sb = const.tile([128, KT * F], BF)
        wo_sb = const.tile([128, 2 * D], FPR)
        cos_sb = const.tile([128, S], FP)
        sin_sb = const.tile([128, S], FP)
        ones64 = const.tile([128, 64], FP)
        nc.vector.memset(ones64[:], 1.0)
        # single-DMA weight loads (multiple DMAs into one tile would attach
        # too many sem waits to the first fused-LDW matmul for walrus);
        # issued from Pool/SWDGE so ACT/SP queues stay clear
        nc.gpsimd.dma_start(wq_sb[:].rearrange("p (k f) -> p k f", k=KT),
                            wqT[:].rearrange("(k p) f -> p k f", p=128))
        nc.gpsimd.dma_start(wk_sb[:].rearrange("p (k f) -> p k f", k=KT),
                            wkT[:].rearrange("(k p) f -> p k f", p=128))
        nc.gpsimd.dma_start(wqr_sb[:].rearrange("p (k f) -> p k f", k=KT),
                            wqrT[:].rearrange("(k p) f -> p k f", p=128))
        nc.gpsimd.dma_start(wkr_sb[:].rearrange("p (k f) -> p k f", k=KT),
                            wkrT[:].rearrange("(k p) f -> p k f", p=128))
        nc.gpsimd.dma_start(cos_sb[:], cosd[:])
        nc.gpsimd.dma_start(sin_sb[:], sind[:])
        if mask_mode == "causal":
            tri_sb = const.tile([128, 256], FP)

        # persistent activations: [p, t*S + s] layouts (t-tile 0: heads 0,1;
        # t-tile 1: heads 2,3 of the group)
        qt_sb = qk.tile([128, 2 * S], FPR)
        kt_sb = qk.tile([128, 2 * S], FPR)
        # V in [s, f] layout with a ones column per head: 65-wide head blocks
        v_sb = qk.tile([128, ST * HPG * 65], FPR)
        ctxn_sb = qk.tile([128, 2 * S], FPR)

        # xv pool outlives phase 1: V projection is fused into the
        # attention stream (one s-tile just ahead of the scores that use it)
        xvpool = ctx.enter_context(tc.tile_pool(name="xvs", bufs=4))

        # ---------------- phase 1: projections + RoPE ----------------
        with ExitStack() as pctx:
            xpool = pctx.enter_context(tc.tile_pool(name="xs", bufs=16))
            rpool = pctx.enter_context(tc.tile_pool(name="rope", bufs=2))
            pps = pctx.enter_context(tc.tile_pool(name="pps", bufs=4, space="PSUM"))

            # issue all xq then all xk loads upfront: K's tiles are resident
            # the moment Q's PSUM accumulators free up — attention gates on
            # K-t0's RoPE, so everything on that path is prioritized
            x_tiles = {}
            for nm, x_d in (("q", xqT), ("k", xkT)):
                for k in range(KT):
                    xt = xpool.tile([128, S], BF, tag="x", name=f"x{nm}{k}")
                    nc.sync.dma_start(xt[:], x_d[k * 128:(k + 1) * 128, :])
                    x_tiles[(nm, k)] = xt
            # late consts: wv/tri/wo aren't needed before ~30us — issuing
            # them after the x loads keeps their transfers out of the DMA
            # queue ahead of the critical xk arrival
            nc.sync.dma_start(wv_sb[:].rearrange("p (k f) -> p k f", k=KT),
                              wvT[:].rearrange("(k p) f -> p k f", p=128))
            if mask_mode == "causal":
                nc.sync.dma_start(tri_sb[:], triD[:])
            nc.sync.dma_start(wo_sb[:].rearrange("p (t e) -> p t e", t=2),
                              woT[:].rearrange("(t p) e -> p t e", p=128))

            def rope(dst_sb, t, eng):
                """eng: DVE (critical path) or Pool (off-path t1 tiles)."""
                lo, hi = t * S, (t + 1) * S
                rot = rpool.tile([128, S], FPR, tag="rot")
                # rotate-half across partitions: [0:32]<-[32:64],
                # [32:64]<-[0:32], [64:96]<-[96:128], [96:128]<-[64:96]
                for dst0, src0 in ((0, 32), (32, 0), (64, 96), (96, 64)):
                    nc.gpsimd.dma_start(rot[dst0:dst0 + 32, :],
                                        dst_sb[src0:src0 + 32, lo:hi])
                eng.tensor_mul(rot[:], rot[:], sin_sb[:])
                eng.tensor_mul(dst_sb[:, lo:hi], dst_sb[:, lo:hi], cos_sb[:])
                eng.tensor_add(dst_sb[:, lo:hi], dst_sb[:, lo:hi], rot[:])

            def dual_t(nm, w_sb, wr_sb, dst_sb, t):
                """DUAL projection of one t-pair: W x AND (P W) x with the
                rotate-half permutation P folded into a second host-side
                weight copy, so RoPE needs no serialized rotate DMAs (they
                cost 3.2us EACH in a WAW chain): dst = main*cos + rot*sin
                straight from PSUM. main+rot = 4 tiles = all 8 banks."""
                m = [pps.tile([128, 1024], FP, tag="pj", name=f"m{nm}{t}{i}")
                     for i in range(2)]
                r = [pps.tile([128, 1024], FP, tag="pj", name=f"r{nm}{t}{i}")
                     for i in range(2)]
                for k in range(KT):
                    xt = x_tiles[(nm, k)]
                    for sc in range(4):
                        _mm(nc, m[sc // 2][:, (sc % 2) * 512:(sc % 2) * 512 + 512],
                            w_sb[:, k * F + t * 128: k * F + (t + 1) * 128],
                            xt[:, sc * 512:(sc + 1) * 512],
                            start=(k == 0), stop=(k == KT - 1))
                        _mm(nc, r[sc // 2][:, (sc % 2) * 512:(sc % 2) * 512 + 512],
                            wr_sb[:, k * F + t * 128: k * F + (t + 1) * 128],
                            xt[:, sc * 512:(sc + 1) * 512],
                            start=(k == 0), stop=(k == KT - 1))
                scr = rpool.tile([128, S], FPR, tag="rot")
                for half in range(2):
                    lo = t * S + half * 1024
                    co = half * 1024
                    nc.vector.tensor_mul(scr[:, co:co + 1024], r[half][:],
                                         sin_sb[:, co:co + 1024])
                    nc.vector.tensor_mul(dst_sb[:, lo:lo + 1024], m[half][:],
                                         cos_sb[:, co:co + 1024])
                    nc.vector.tensor_add(dst_sb[:, lo:lo + 1024],
                                         dst_sb[:, lo:lo + 1024],
                                         scr[:, co:co + 1024])

            def classic_t(nm, w_sb, dst_sb, t):
                """Classic projection of one t-pair (2 tiles = 4 banks) +
                rotate-DMA RoPE; used for the t1 halves whose rot chains
                overlap attention off the critical path."""
                ps = [pps.tile([128, 1024], FP, tag="pj", name=f"c{nm}{t}{i}")
                      for i in range(2)]
                for k in range(KT):
                    xt = x_tiles[(nm, k)]
                    for sc in range(4):
                        _mm(nc, ps[sc // 2][:, (sc % 2) * 512:(sc % 2) * 512 + 512],
                            w_sb[:, k * F + t * 128: k * F + (t + 1) * 128],
                            xt[:, sc * 512:(sc + 1) * 512],
                            start=(k == 0), stop=(k == KT - 1))
                for half in range(2):
                    nc.scalar.copy(
                        dst_sb[:, t * S + half * 1024: t * S + (half + 1) * 1024],
                        ps[half][:])
                rope(dst_sb, t, nc.vector)

            # t0 halves (heads 0,1) dual-projected — first exp gates on them
            dual_t("q", wq_sb, wqr_sb, qt_sb, 0)
            dual_t("k", wk_sb, wkr_sb, kt_sb, 0)
            # t1 halves classic: their rot chains run under attention
            classic_t("q", wq_sb, qt_sb, 1)
            classic_t("k", wk_sb, kt_sb, 1)

        # V: out[s_tile, f] layout via column-slab x loads, one s-tile at a
        # time, emitted from inside the attention loop just ahead of use so
        # the PE stream reaches the first scores ~6us earlier
        _v_done = set()
        ctx_pool_ref = [None]

        def emit_v(st):
            if st in _v_done or st >= ST:
                return
            _v_done.add(st)
            xslab = xvpool.tile([128, KT * 128], BF, tag="xv", name=f"xv{st}")
            nc.sync.dma_start(
                xslab[:], xvS[st * 128:(st + 1) * 128, :])
            pv = ctx_pool_ref[0].tile([128, 1024], FP, tag="ctx", name=f"pv{st}")
            for k in range(KT):
                _mm(nc, pv[:, 0:256], xslab[:, k * 128:(k + 1) * 128],
                    wv_sb[:, k * F:(k + 1) * F],
                    start=(k == 0), stop=(k == KT - 1))
            c0 = st * HPG * 65
            dstv = v_sb[:, c0:c0 + HPG * 65].rearrange(
                "p (h c) -> p h c", h=HPG)[:, :, 0:64]
            srcv = pv[:, 0:256].rearrange("p (h c) -> p h c", h=HPG)
            nc.scalar.copy(dstv, srcv)

        # ---------------- phase 2: attention ----------------
        with ExitStack() as actx:
            sc_ps = actx.enter_context(tc.tile_pool(name="scps", bufs=2, space="PSUM"))
            ctx_ps = actx.enter_context(tc.tile_pool(name="ctxps", bufs=2, space="PSUM"))
            ctx_pool_ref[0] = ctx_ps
            epool = actx.enter_context(tc.tile_pool(name="exp", bufs=8))
            npool = actx.enter_context(tc.tile_pool(name="norm", bufs=2))
            opool = actx.enter_context(tc.tile_pool(name="ost", bufs=4))
            if mask_mode == "general":
                bpool = actx.enter_context(tc.tile_pool(name="bias", bufs=2))

            ones_ap = v_sb[:].rearrange("p (b c) -> p b c", c=65)[:, :, 64:65]
            nc.vector.tensor_copy(ones_ap, ones64[:].rearrange("p (b o) -> p b o", o=1))

            # t0 heads (0,1) of both q-chunks first: the t1 RoPE finishes
            # while they run, so ACT never idles waiting for heads 2,3
            for qc, h in ((0, 0), (0, 1), (1, 0), (1, 1),
                          (0, 2), (0, 3), (1, 2), (1, 3)):
                if True:
                    t, po = h // 2, (h % 2) * 64
                    kt_hi = 8 * qc + 8 if mask_mode == "causal" else ST
                    last_b0 = min(kt_hi - 1, 8 * qc + 3) if mask_mode == "causal" else ST - 1
                    ctx_t = ctx_ps.tile([128, QCW], FP, tag="ctx")
                    qbase = t * S + qc * QCW

                    def emit_pv(e_t, kt, a0, a1):
                        vcol = kt * HPG * 65 + h * 65
                        if a0 < 512:
                            _mm(nc, ctx_t[0:65, a0:512], v_sb[:, vcol:vcol + 65],
                                e_t[:, a0:512],
                                start=(kt == 0), stop=(kt == last_b0))
                        _mm(nc, ctx_t[0:65, a1:QCW], v_sb[:, vcol:vcol + 65],
                            e_t[:, a1:QCW],
                            start=(kt == 0), stop=(kt == kt_hi - 1))

                    # software pipeline: PV(kt-3) is emitted AFTER scores(kt)
                    # so the in-order PE stream never stalls waiting for
                    # exp(kt-3) with scores work available
                    if h == 0:
                        emit_v(8 * qc)
                        emit_v(8 * qc + 1)
                    pend = []
                    for kt in range(kt_hi):
                        if h == 0:
                            emit_v(kt + 2)
                        if mask_mode == "causal":
                            j0, a0, a1 = _causal_ranges(qc, kt)
                        else:
                            j0, a0, a1 = 0, 0, 512
                        kcol = t * S + kt * 128
                        s_ps = sc_ps.tile([128, QCW], FP, tag="sc")
                        if a0 < 512:
                            _mm(nc, s_ps[:, a0:512],
                                kt_sb[po:po + 64, kcol:kcol + 128],
                                qt_sb[po:po + 64, qbase + a0:qbase + 512],
                                start=True, stop=True)
                        _mm(nc, s_ps[:, a1:QCW],
                            kt_sb[po:po + 64, kcol:kcol + 128],
                            qt_sb[po:po + 64, qbase + a1:qbase + QCW],
                            start=True, stop=True)
                        if mask_mode == "general":
                            bt = bpool.tile([128, QCW], FP, tag="bt")
                            nc.sync.dma_start(
                                bt[:], biasT[kt * 128:(kt + 1) * 128,
                                             qc * QCW:(qc + 1) * QCW])
                            nc.vector.tensor_add(s_ps[:], s_ps[:], bt[:])
                        e_t = epool.tile([128, QCW], FPR, tag="e")
                        # exp from the first COMPUTED column (a-range) — the
                        # [a:j0) strip holds real but masked scores, zeroed by
                        # the tri multiply below
                        ax = a0 if a0 < 512 else a1
                        nc.scalar.activation(e_t[:, ax:QCW], s_ps[:, ax:QCW], EXP)
                        if mask_mode == "causal" and kt * 128 >= qc * QCW:
                            if j0 > ax:
                                nc.vector.tensor_mul(e_t[:, ax:ax + 256],
                                                     e_t[:, ax:ax + 256],
                                                     tri_sb[:])
                            else:
                                nc.vector.tensor_mul(e_t[:, j0:j0 + 128],
                                                     e_t[:, j0:j0 + 128],
                                                     tri_sb[:, 128:256])
                        pend.append((e_t, kt, a0, a1))
                        if len(pend) > 3:
                            emit_pv(*pend.pop(0))
                    for p_ in pend:
                        emit_pv(*p_)
                    # normalize: rows 0:64 are ctx^T, row 64 is sum(exp)
                    r_sb = npool.tile([1, QCW], FPR, tag="r")
                    with nc.allow_low_precision(reason="float32r == fp32 width"):
                        nc.vector.reciprocal(r_sb[:], ctx_t[64:65, :])
                    # broadcast 1/sum across partitions via a 0-stride-source
                    # SBUF->SBUF DMA (keeps PE/DVE/ACT out of the per-head
                    # normalize chain)
                    rb_sb = npool.tile([64, QCW], FPR, tag="rb")
                    nc.sync.dma_start(
                        rb_sb[:],
                        r_sb[:].rearrange("p (o s) -> p o s", o=1)
                               .broadcast_to([1, 64, QCW]))
                    nc.vector.tensor_mul(
                        ctxn_sb[po:po + 64, t * S + qc * QCW: t * S + (qc + 1) * QCW],
                        ctx_t[0:64, :], rb_sb[:])

            # output projection AFTER both attention chunks: the scheduler
            # interleaves these PE-only tiles into the exp-bound attention
            # stream's gaps instead of stalling ACT mid-sequence
            for st in range(ST):
                o_ps = sc_ps.tile([128, QCW], FP, tag="sc", name="ops")
                for ec in range(2):
                    for ft in range(2):
                        _mm(nc, o_ps[:, ec * 512:(ec + 1) * 512],
                            ctxn_sb[:, ft * S + st * 128: ft * S + (st + 1) * 128],
                            wo_sb[:, ft * D + ec * 512: ft * D + (ec + 1) * 512],
                            start=(ft == 0), stop=(ft == 1))
                o_sb = opool.tile([128, QCW], BF, tag="o")
                if st % 2 == 0:
                    nc.vector.tensor_copy(o_sb[:], o_ps[:])
                else:
                    nc.scalar.copy(o_sb[:], o_ps[:])
                nc.gpsimd.dma_start(
                    outp[st * 128:(st + 1) * 128, :], o_sb[:])
    _hoist_waits(nc)
    return nc


def _get_nc(mask_mode):
    if mask_mode not in _nc_cache:
        _nc_cache[mask_mode] = build_nc(mask_mode)
    return _nc_cache[mask_mode]


def _rope_tables():
    """cos/sin tables in [128, S] layout (64-row block tiled twice); sin is
    sign-folded for the rotate-half term."""
    inv_freq = (1.0 / (10000.0 ** (np.arange(0, DK, 2, dtype=np.float32) / np.float32(DK)))).astype(np.float32)
    t = np.arange(S, dtype=np.float32)
    freqs = np.outer(t, inv_freq).astype(np.float32)      # (S, 32)
    emb = np.concatenate([freqs, freqs], axis=-1)         # (S, 64)
    cos64 = np.cos(emb).T.astype(np.float32)              # (64, S)
    sin64 = np.sin(emb).T.astype(np.float32)
    sin64s = sin64.copy()
    sin64s[0:32] = -sin64[0:32]
    cos128 = np.ascontiguousarray(np.tile(cos64, (2, 1)))
    sin128 = np.ascontiguousarray(np.tile(sin64s, (2, 1)))
    return cos128, sin128


def _mask_mode(m2d):
    if (m2d != 0).all():
        return "full"
    if np.array_equal(m2d != 0, np.tril(np.ones((S, S), dtype=bool))):
        return "causal"
    return "general"


def _prepare(inputs):
    q = np.asarray(inputs["query"], dtype=np.float32)
    k = np.asarray(inputs["key"], dtype=np.float32)
    v = np.asarray(inputs["value"], dtype=np.float32)
    mask = np.asarray(inputs["mask"])
    Wq = np.asarray(inputs["W_q"], dtype=np.float32)
    Wk = np.asarray(inputs["W_k"], dtype=np.float32)
    Wv = np.asarray(inputs["W_v"], dtype=np.float32)
    Wo = np.asarray(inputs["W_o"], dtype=np.float32)

    modes = [_mask_mode(mask[b, 0]) for b in range(NB)]
    if all(m == "causal" for m in modes):
        mode = "causal"
    elif all(m == "full" for m in modes):
        mode = "full"
    else:
        mode = "general"
    nc = _get_nc(mode)

    cos128, sin128 = _rope_tables()
    scale = np.float32(1.0 / np.sqrt(DK))
    if mode == "causal":
        kk = np.arange(128)[:, None]
        qq = np.arange(128)[None, :]
        tri = np.where(kk <= qq, np.float32(1.0), np.float32(0.0))
        triD = np.concatenate([np.zeros((128, 128), np.float32), tri],
                              axis=1).astype(np.float32)

    import ml_dtypes
    bf16 = ml_dtypes.bfloat16

    xT = {}
    biasTs = {}
    for b in range(NB):
        # xvS[st*128+p, k*128+c] = v.T[k*128+p, st*128+c]: each s-tile's
        # column slab is contiguous so the device loads it in one DMA
        vT = v[b].T
        xvS = np.ascontiguousarray(
            vT.reshape(KT, 128, ST, 128).transpose(2, 1, 0, 3)
              .reshape(ST * 128, KT * 128))
        xT[b] = (np.ascontiguousarray(q[b].T).astype(bf16),
                 np.ascontiguousarray(k[b].T).astype(bf16),
                 xvS.astype(bf16))
        if mode == "general":
            biasTs[b] = np.where(mask[b, 0].T != 0, np.float32(0.0),
                                 np.float32(NEG)).astype(np.float32)

    in_maps = []
    for core in range(8):
        b, g = divmod(core, 4)
        rows = slice(g * F, (g + 1) * F)
        m = {
            "xqT": xT[b][0], "xkT": xT[b][1], "xvS": xT[b][2],
            "wqT": np.ascontiguousarray((Wq[rows] * scale).T).astype(bf16),
            "wkT": np.ascontiguousarray(Wk[rows].T).astype(bf16),
            "wqrT": np.ascontiguousarray((Wq[rows] * scale).T[:, _ROT_PERM]).astype(bf16),
            "wkrT": np.ascontiguousarray(Wk[rows].T[:, _ROT_PERM]).astype(bf16),
            "wvT": np.ascontiguousarray(Wv[rows].T).astype(bf16),
            "woT": np.ascontiguousarray(Wo[:, rows].T),
            "cosS": cos128, "sinS": sin128,
        }
        if mode == "general":
            m["biasT"] = biasTs[b]
        if mode == "causal":
            m["triD"] = triD
        in_maps.append(m)
    return nc, in_maps


def _gather(res):
    out = np.zeros((NB, S, D), dtype=np.float32)
    for core in range(8):
        out[core // 4] += np.asarray(res.results[core]["outp"],
                                     dtype=np.float32)
    return out


def kernel(**inputs):
    from concourse import bass_utils

    nc, in_maps = _prepare(inputs)
    res = bass_utils.run_bass_kernel_spmd(nc, in_maps, core_ids=list(range(8)))
    return _gather(res)


def run_traced(**inputs):
    """Run once with NTFF tracing; returns (out, exec_time_ns, raw results)."""
    from concourse import bass_utils

    nc, in_maps = _prepare(inputs)
    res = bass_utils.run_bass_kernel_spmd(nc, in_maps, core_ids=list(range(8)),
                                          trace=True)
    return _gather(res), res.exec_time_ns, res
